# revision 22
# baseline (speedup 1.0000x reference)
"""Trainium2 Bass kernel for nn_BlockAttnRes (block-softmax residual net).

Shapes: embedding [8, 8192, 128] f32, L=16 layers, BLOCK_SIZE=4.
Sharding: batch dim B=8 across 8 cores (1 batch row / core = 8192 tokens).

Per-core: tokens-on-partitions ("row") bf16 state resident in SBUF.
6 state slots: slot0 = emb, slot 1+g = partial of group g (becomes block g+1
at commit). Loop: For_i over token tiles (F=512 tokens = 4 blocks of 128),
python-unrolled 16 layers inside. Key techniques:
  - static-source logits for all layers precomputed at source creation
    (PE f32r matmuls + tiny chunk transposes)
  - partial-source stats per layer via fused scalar_tensor_tensor accums
  - softmax-exp via tanh identity e^t=(1+T)/(1-T) (gelu ACT table set only)
  - rsqrt via int bit-trick seed + 2 Newton iterations (DVE only)
  - weighted sums via per-block fused stt chains
  - LayerNorm affine folded into W1' = diag(g)@W1, b1' = b1 + ln_b@W1 (host)
  - MLP: PE transposes to col layout, bf16 matmuls, ACT gelu fused bias
  - partial accumulated in PSUM by f32 transpose-matmuls (start=False accum)
"""
import contextlib
import ctypes
import sys
import types
from contextlib import ExitStack

sys.path.insert(0, "/opt/trn_rl_repo")


def _install_ntff_hook():
    """Provide antenv.axon_hooks (missing in the trimmed repo) so
    run_bass_kernel_spmd(trace=True) can collect NTFF profiles."""
    if "antenv.axon_hooks" in sys.modules:
        return
    try:
        lib = ctypes.CDLL("/opt/axon/libaxon_pjrt.so")
    except OSError:
        return
    if not hasattr(lib, "axon_start_nrt_profile"):
        hook = None
    else:
        lib.axon_start_nrt_profile.argtypes = [
            ctypes.POINTER(ctypes.c_int64), ctypes.c_size_t]
        lib.axon_start_nrt_profile.restype = ctypes.c_int64
        lib.axon_stop_nrt_profile.argtypes = [ctypes.c_char_p]
        lib.axon_stop_nrt_profile.restype = ctypes.c_int64

        @contextlib.contextmanager
        def hook(output_dir, device_ids):
            import jax
            jax.devices()
            if device_ids:
                ids = (ctypes.c_int64 * len(device_ids))(*device_ids)
                rc = lib.axon_start_nrt_profile(ids, len(device_ids))
            else:
                rc = lib.axon_start_nrt_profile(None, 0)
            if rc != 0:
                raise RuntimeError(f"axon_start_nrt_profile rc={rc}")
            try:
                yield
            finally:
                n = lib.axon_stop_nrt_profile(str(output_dir).encode())
                print(f"profile: {n} file(s) -> {output_dir}", file=sys.stderr)

    mod = types.ModuleType("antenv.axon_hooks")
    mod.get_axon_ntff_profile_hook = lambda: hook
    mod.set_axon_ntff_profile_hook = lambda h: None
    sys.modules["antenv.axon_hooks"] = mod

import numpy as np
import ml_dtypes

import concourse.bacc as bacc
import concourse.bass as bass
import concourse.mybir as mybir
from concourse.bass_utils import run_bass_kernel_spmd
from concourse.tile import TileContext
from concourse.masks import make_identity

F32 = mybir.dt.float32
BF16 = mybir.dt.bfloat16
F32R = mybir.dt.float32r
I32 = mybir.dt.int32
ALU = mybir.AluOpType
AF = mybir.ActivationFunctionType
AX = mybir.AxisListType

L = 16
GROUP = 4
D = 128
NBLK = 4                 # 128-token blocks per tile
F = NBLK * 128           # tokens per tile
EPS_RMS = 1e-8
EPS_LN = 1e-5
MAGIC = 0x5F3759DF
N_CORES = 8

_CACHE = {}


def _bcast(ap, n):
    """Append a stride-0 inner free dim of size n to an AP."""
    return bass.AP(tensor=ap.tensor, offset=ap.offset,
                   ap=list(ap.ap) + [[0, n]])


def _newton_rsqrt(nc, pool, x, shape, iters=2):
    """y = rsqrt(x) for x [128, *shape] f32 tile (positive). Returns y tile."""
    y = pool.tile([128] + list(shape), F32, tag="nw_y", name="nw_y")
    xi = x.bitcast(I32)
    yi = y.bitcast(I32)
    nc.vector.tensor_scalar(out=yi[:], in0=xi[:], scalar1=1, scalar2=0,
                            op0=ALU.logical_shift_right,
                            op1=ALU.logical_shift_right)
    nc.vector.tensor_scalar(out=yi[:], in0=yi[:], scalar1=-1, scalar2=MAGIC,
                            op0=ALU.mult, op1=ALU.add)
    t = pool.tile([128] + list(shape), F32, tag="nw_t", name="nw_t")
    for _ in range(iters):
        nc.vector.tensor_mul(t[:], y[:], y[:])
        nc.vector.scalar_tensor_tensor(out=t[:], in0=t[:], scalar=-0.5,
                                       in1=x[:], op0=ALU.mult, op1=ALU.mult)
        nc.vector.scalar_tensor_tensor(out=y[:], in0=t[:], scalar=1.5,
                                       in1=y[:], op0=ALU.add, op1=ALU.mult)
    return y


def build(tiles_per_core=16, dbg=False):
    nc = bacc.Bacc("TRN2", target_bir_lowering=False)
    n_tok = tiles_per_core * F

    emb = nc.dram_tensor("emb", [n_tok, D], F32, kind="ExternalInput")
    w_t = nc.dram_tensor("w_t", [D, L], F32, kind="ExternalInput")
    w_f = nc.dram_tensor("w_f", [L, D], F32, kind="ExternalInput")
    w1p = nc.dram_tensor("w1p", [D, L * 2 * 128], BF16, kind="ExternalInput")
    b1p = nc.dram_tensor("b1p", [128, 2 * L], F32, kind="ExternalInput")
    w2p = nc.dram_tensor("w2p", [128, L * 2 * D], BF16, kind="ExternalInput")
    out = nc.dram_tensor("out", [n_tok, D], F32, kind="ExternalOutput")
    if dbg:
        dbgH = nc.dram_tensor("dbgH", [L, 128, NBLK, D], BF16, kind="ExternalOutput")
        dbgW = nc.dram_tensor("dbgW", [L, 128, NBLK, 5], F32, kind="ExternalOutput")
        dbgS = nc.dram_tensor("dbgS", [128, 5, NBLK, L], F32, kind="ExternalOutput")

    emb_v = emb.rearrange("(T b p) d -> T p b d", b=NBLK, p=128)
    out_v = out.rearrange("(T b p) d -> T p b d", b=NBLK, p=128)

    with TileContext(nc) as tc, ExitStack() as es:
        cst = es.enter_context(tc.tile_pool(name="cst", bufs=1))
        ident16 = cst.tile([128, 128], BF16)
        make_identity(nc, ident16[:])
        identf = cst.tile([128, 128], F32)
        make_identity(nc, identf[:])
        identr = cst.tile([128, 128], F32R)
        nc.vector.tensor_copy(identr[:], identf[:])

        wallT = cst.tile([128, L], F32)
        nc.sync.dma_start(wallT[:], w_t[:])
        wallT_r = cst.tile([128, L], F32R)
        nc.vector.tensor_copy(wallT_r[:], wallT[:])
        onesf = cst.tile([128, 1], F32)
        nc.vector.memset(onesf[:], 1.0)
        ones_r = cst.tile([128, 1], F32R)
        nc.vector.tensor_copy(ones_r[:], onesf[:])

        # w broadcast across partitions: [128, L, D] f32
        wrep = cst.tile([128, L, D], F32)
        wfa = w_f[:]
        nc.gpsimd.dma_start(
            out=wrep[:],
            in_=bass.AP(tensor=wfa.tensor, offset=wfa.offset,
                        ap=[[0, 128]] + list(wfa.ap)))

        w1p_sb = cst.tile([128, L, 2, 128], BF16)
        nc.sync.dma_start(w1p_sb[:], w1p[:].rearrange(
            "d (l h m) -> d l h m", l=L, h=2))
        b1p_sb = cst.tile([128, 2 * L], F32)
        nc.sync.dma_start(b1p_sb[:], b1p[:])
        w2p_sb = cst.tile([128, L, 2, D], BF16)
        nc.sync.dma_start(w2p_sb[:], w2p[:].rearrange(
            "m (l k d) -> m l k d", l=L, k=2))

        zero16 = cst.tile([128, NBLK, D], BF16)
        nc.vector.memset(zero16[:], 0.0)

        sp = es.enter_context(tc.tile_pool(name="state", bufs=4))
        big = es.enter_context(tc.tile_pool(name="big", bufs=4))
        sml = es.enter_context(tc.tile_pool(name="sml", bufs=8))
        nwp = es.enter_context(tc.tile_pool(name="nw", bufs=8))
        pp_xnT = es.enter_context(tc.tile_pool(name="pp_xnT", bufs=1, space="PSUM"))
        pp_big = es.enter_context(tc.tile_pool(name="pp_big", bufs=3, space="PSUM"))
        pp_par = es.enter_context(tc.tile_pool(name="pp_par", bufs=4, space="PSUM"))

        def creation_stats(slot_buf, sdots, s_idx):
            srcT_ps = pp_xnT.tile([128, F], F32R, tag="xnT_ps", name="srcT_ps")
            for blk in range(NBLK):
                nc.tensor.matmul(srcT_ps[:, blk * 128:(blk + 1) * 128],
                                 slot_buf[:, blk, :], identr[:],
                                 is_transpose=True, start=True, stop=True,
                                 skip_group_check=True)
            srcT_r = big.tile([128, F], F32R, tag="srcT_r")
            nc.vector.tensor_copy(srcT_r[:], srcT_ps[:])
            sq_r = big.tile([128, F], F32R, tag="sq_r")
            nc.scalar.activation(sq_r[:], srcT_r[:], AF.Square)
            dots_ps = pp_big.tile([33, F], F32, tag="big_ps")
            nc.tensor.matmul(dots_ps[0:16, :], wallT_r[:], srcT_r[:],
                             start=True, stop=True, skip_group_check=True)
            ms_ps = pp_big.tile([1, F], F32, tag="big_ps")
            nc.tensor.matmul(ms_ps[:], ones_r[:], sq_r[:],
                             start=True, stop=True, skip_group_check=True)
            dots_sb = big.tile([33, F], F32, tag="dots_sb")
            nc.vector.tensor_copy(dots_sb[0:16, :], dots_ps[0:16, :])
            nc.vector.tensor_copy(dots_sb[32:33, :], ms_ps[:])
            statT_ps = pp_big.tile([128, NBLK, 33], F32, tag="big_ps")
            for c in range(NBLK):
                nc.tensor.matmul(statT_ps[:, c, :],
                                 dots_sb[:, c * 128:(c + 1) * 128],
                                 identf[0:33, 0:33],
                                 is_transpose=True, start=True, stop=True,
                                 skip_group_check=True)
            stats_sb = sml.tile([128, NBLK, 33], F32, tag="stats_sb")
            nc.vector.tensor_copy(stats_sb[:], statT_ps[:])
            xs = sml.tile([128, NBLK], F32, tag="xs_cr")
            nc.vector.tensor_scalar(out=xs[:], in0=stats_sb[:, :, 32],
                                    scalar1=1.0 / D, scalar2=EPS_RMS,
                                    op0=ALU.mult, op1=ALU.add)
            rms = _newton_rsqrt(nc, nwp, xs, xs.shape[1:])
            nc.vector.scalar_tensor_tensor(
                out=sdots[:, s_idx, :, :], in0=stats_sb[:, :, 0:16],
                scalar=1.0, in1=_bcast(rms[:], L),
                op0=ALU.bypass, op1=ALU.mult)

        def tile_start(it, k, sh):
            st = {"it": it, "k": k, "sh": sh}
            st["slots"] = sp.tile([128, 6, NBLK, D], F32R, tag="slots", name="slots")
            st["trash"] = sp.tile([128, NBLK, D], BF16, tag="trash", name="trash")
            st["sdots"] = sh["sdots_all"][:, k]
            emb_st = big.tile([128, NBLK, D], F32, tag="emb_st")
            nc.sync.dma_start(emb_st[:], emb_v[bass.ds(it, 1)])
            nc.vector.tensor_copy(st["slots"][:, 0, :, :], emb_st[:])
            st["partial_ps"] = pp_par.tile([128, NBLK, D], F32, tag="par", name="par")
            return st

        def creation_pre(st, slot_buf, s_idx, sh):
            """Per-stream creation: transposes + stat matmuls -> stats_sb_all."""
            k = st["k"]
            srcT_ps = pp_xnT.tile([128, F], F32R, tag="xnT_ps", name="srcT_ps")
            for blk in range(NBLK):
                nc.tensor.matmul(srcT_ps[:, blk * 128:(blk + 1) * 128],
                                 slot_buf[:, blk, :], identr[:],
                                 is_transpose=True, start=True, stop=True,
                                 skip_group_check=True)
            srcT_r = big.tile([128, F], F32R, tag="srcT_r")
            nc.vector.tensor_copy(srcT_r[:], srcT_ps[:])
            sq_r = big.tile([128, F], F32R, tag="sq_r")
            nc.scalar.activation(sq_r[:], srcT_r[:], AF.Square)
            dots_ps = pp_big.tile([33, F], F32, tag="big_ps", name="dots_ps")
            nc.tensor.matmul(dots_ps[0:16, :], wallT_r[:], srcT_r[:],
                             start=True, stop=True, skip_group_check=True)
            ms_ps = pp_big.tile([1, F], F32, tag="big_ps", name="ms_ps")
            nc.tensor.matmul(ms_ps[:], ones_r[:], sq_r[:],
                             start=True, stop=True, skip_group_check=True)
            dots_sb = big.tile([33, F], F32, tag="dots_sb")
            nc.vector.tensor_copy(dots_sb[0:16, :], dots_ps[0:16, :])
            nc.vector.tensor_copy(dots_sb[32:33, :], ms_ps[:])
            statT_ps = pp_big.tile([128, NBLK, 33], F32, tag="big_ps",
                                   name="statT_ps")
            for c in range(NBLK):
                nc.tensor.matmul(statT_ps[:, c, :],
                                 dots_sb[:, c * 128:(c + 1) * 128],
                                 identf[0:33, 0:33],
                                 is_transpose=True, start=True, stop=True,
                                 skip_group_check=True)
            nc.vector.tensor_copy(sh["stats_sb_all"][:, k], statT_ps[:])

        def creation_finish(sts, s_idx, sh):
            """Batched across streams: rms + scaled dots."""
            ns_ = len(sts)
            stats = sh["stats_sb_all"]
            xs = sml.tile([128, NS, NBLK], F32, tag="xs_cr")
            nc.vector.tensor_scalar(out=xs[:, 0:ns_], in0=stats[:, 0:ns_, :, 32],
                                    scalar1=1.0 / D, scalar2=EPS_RMS,
                                    op0=ALU.mult, op1=ALU.add)
            rms = _newton_rsqrt(nc, nwp, xs, (NS, NBLK))
            r_ap = rms[:, 0:ns_]
            nc.vector.scalar_tensor_tensor(
                out=sh["sdots_all"][:, 0:ns_, s_idx, :, :],
                in0=stats[:, 0:ns_, :, 0:16],
                scalar=1.0, in1=_bcast(r_ap, L),
                op0=ALU.bypass, op1=ALU.mult)

        def emit_layer_quad(sts, l, sh):
            ns_ = len(sts)
            g, j = l // GROUP, l % GROUP
            nsrc = g + 1
            has_p = j > 0
            n = nsrc + (1 if has_p else 0)
            sdots_all = sh["sdots_all"]

            E_T = sml.tile([128, NS, NBLK, 5], F32, tag="E_T")
            e_ap = E_T[:]
            statics_out = bass.AP(
                tensor=e_ap.tensor, offset=e_ap.offset,
                ap=[e_ap.ap[0], [NBLK * 5, ns_], [1, nsrc], [5, NBLK]])
            nc.scalar.activation(out=statics_out,
                                 in_=sdots_all[:, 0:ns_, 0:nsrc, :, l],
                                 func=AF.Tanh, scale=0.5)

            if has_p:
                dotp = sml.tile([128, NS, NBLK], F32, tag="dotp")
                ssqp = sml.tile([128, NS, NBLK], F32, tag="ssqp")
                for st in sts:
                    k = st["k"]
                    p_ps = st["slots"][:, g + 1, :, :]
                    for blk in range(NBLK):
                        nc.vector.scalar_tensor_tensor(
                            out=st["trash"][:, blk, :], in0=p_ps[:, blk, :],
                            scalar=1.0, in1=wrep[:, l, :],
                            op0=ALU.bypass, op1=ALU.mult,
                            accum_out=dotp[:, k, blk:blk + 1])
                        nc.scalar.activation(
                            out=st["trash"][:, blk, :], in_=p_ps[:, blk, :],
                            func=AF.Square,
                            accum_out=ssqp[:, k, blk:blk + 1])
                xp = sml.tile([128, NS, NBLK], F32, tag="xp")
                nc.vector.tensor_scalar(out=xp[:, 0:ns_], in0=ssqp[:, 0:ns_],
                                        scalar1=1.0 / D, scalar2=EPS_RMS,
                                        op0=ALU.mult, op1=ALU.add)
                rmsp = _newton_rsqrt(nc, nwp, xp, (NS, NBLK), iters=1)
                lp = sml.tile([128, NS, NBLK], F32, tag="lp")
                nc.vector.tensor_mul(lp[:, 0:ns_], dotp[:, 0:ns_],
                                     rmsp[:, 0:ns_])
                nc.scalar.activation(out=E_T[:, 0:ns_, :, nsrc],
                                     in_=lp[:, 0:ns_],
                                     func=AF.Tanh, scale=0.5)

            Ev = E_T[:, 0:ns_, :, 0:n]
            A = sml.tile([128, NS, NBLK, 5], F32, tag="A")
            nc.vector.tensor_scalar_add(A[:, 0:ns_, :, 0:n], Ev, 1.0)
            Bt = sml.tile([128, NS, NBLK, 5], F32, tag="B")
            nc.vector.tensor_scalar(out=Bt[:, 0:ns_, :, 0:n], in0=Ev,
                                    scalar1=-1.0, scalar2=-1.0,
                                    op0=ALU.mult, op1=ALU.subtract)
            R = sml.tile([128, NS, NBLK, 5], F32, tag="R")
            nc.vector.reciprocal(R[:, 0:ns_, :, 0:n], Bt[:, 0:ns_, :, 0:n])
            E = sml.tile([128, NS, NBLK, 5], F32, tag="E")
            nc.vector.tensor_mul(E[:, 0:ns_, :, 0:n], A[:, 0:ns_, :, 0:n],
                                 R[:, 0:ns_, :, 0:n])
            den = sml.tile([128, NS, NBLK], F32, tag="den")
            nc.vector.tensor_reduce(den[:, 0:ns_], E[:, 0:ns_, :, 0:n],
                                    axis=AX.X, op=ALU.add)
            rd = sml.tile([128, NS, NBLK], F32, tag="rd")
            nc.vector.reciprocal(rd[:, 0:ns_], den[:, 0:ns_])
            wts = sml.tile([128, NS, NBLK, 5], F32, tag="wts")
            rd_ap = rd[:, 0:ns_]
            nc.vector.scalar_tensor_tensor(
                out=wts[:, 0:ns_, :, 0:n], in0=E[:, 0:ns_, :, 0:n], scalar=1.0,
                in1=_bcast(rd_ap, n), op0=ALU.bypass, op1=ALU.mult)

            last = l == L - 1
            hsum = sml.tile([128, NS, NBLK], F32, tag="hsum")
            hs = []
            for st in sts:
                k = st["k"]
                slots, partial_ps = st["slots"], st["partial_ps"]

                def wsrc(i, blk):
                    return slots[:, i, blk, :]

                h = big.tile([128, NBLK, D], F32 if last else F32R,
                             tag="h_f32" if last else "h", name="h")
                hs.append(h)
                for blk in range(NBLK):
                    if n == 1:
                        nc.vector.tensor_scalar(
                            out=h[:, blk, :], in0=wsrc(0, blk),
                            scalar1=wts[:, k, blk, 0:1], scalar2=0.0,
                            op0=ALU.mult, op1=ALU.add,
                            accum_out=hsum[:, k, blk:blk + 1])
                    else:
                        nc.gpsimd.tensor_scalar(
                            out=h[:, blk, :], in0=wsrc(0, blk),
                            scalar1=wts[:, k, blk, 0:1], scalar2=None,
                            op0=ALU.mult)
                    for i in range(1, n):
                        nc.vector.scalar_tensor_tensor(
                            out=h[:, blk, :], in0=wsrc(i, blk),
                            scalar=wts[:, k, blk, i:i + 1],
                            in1=h[:, blk, :],
                            op0=ALU.mult, op1=ALU.add,
                            accum_out=(hsum[:, k, blk:blk + 1]
                                       if i == n - 1 else None))
                if dbg and k == 0:
                    nc.sync.dma_start(dbgW[l], wts[:, 0])
                    if not last:
                        nc.sync.dma_start(dbgH[l], h[:])
                if last:
                    nc.sync.dma_start(out_v[bass.ds(st["it"], 1)], h[:])
                    if dbg and k == 0:
                        nc.sync.dma_start(dbgS[:], st["sdots"])
            if last:
                return

            hssq = sml.tile([128, NS, NBLK], F32, tag="hssq")
            for st, h in zip(sts, hs):
                k = st["k"]
                for blk in range(NBLK):
                    nc.vector.scalar_tensor_tensor(
                        out=st["trash"][:, blk, :], in0=h[:, blk, :],
                        scalar=1.0, in1=h[:, blk, :],
                        op0=ALU.bypass, op1=ALU.mult,
                        accum_out=hssq[:, k, blk:blk + 1])
            m2 = sml.tile([128, NS, NBLK], F32, tag="m2")
            nc.vector.tensor_mul(m2[:, 0:ns_], hsum[:, 0:ns_], hsum[:, 0:ns_])
            t1 = sml.tile([128, NS, NBLK], F32, tag="t1")
            nc.vector.tensor_scalar(out=t1[:, 0:ns_], in0=hssq[:, 0:ns_],
                                    scalar1=1.0 / D, scalar2=EPS_LN,
                                    op0=ALU.mult, op1=ALU.add)
            xs2 = sml.tile([128, NS, NBLK], F32, tag="xs2")
            nc.vector.scalar_tensor_tensor(
                out=xs2[:, 0:ns_], in0=m2[:, 0:ns_], scalar=-1.0 / (D * D),
                in1=t1[:, 0:ns_], op0=ALU.mult, op1=ALU.add)
            s_ln = _newton_rsqrt(nc, nwp, xs2, (NS, NBLK), iters=1)
            mu = sml.tile([128, NS, NBLK], F32, tag="mu")
            nc.vector.tensor_scalar_mul(mu[:, 0:ns_], hsum[:, 0:ns_], 1.0 / D)

            for st, h in zip(sts, hs):
                k = st["k"]
                slots, partial_ps = st["slots"], st["partial_ps"]
                xn = big.tile([128, NBLK, D], BF16, tag="xn", name="xn")
                for blk in range(NBLK):
                    nc.gpsimd.tensor_scalar(
                        out=xn[:, blk, :], in0=h[:, blk, :],
                        scalar1=mu[:, k, blk:blk + 1],
                        scalar2=s_ln[:, k, blk:blk + 1],
                        op0=ALU.subtract, op1=ALU.mult)
                xnT_ps = pp_xnT.tile([128, F], BF16, tag="xnT_ps", name="xnT_ps")
                for blk in range(NBLK):
                    nc.tensor.matmul(xnT_ps[:, blk * 128:(blk + 1) * 128],
                                     xn[:, blk, :], ident16[:],
                                     is_transpose=True, start=True, stop=True,
                                     skip_group_check=True)
                xnT = big.tile([128, F], BF16, tag="xnT", name="xnT")
                nc.scalar.copy(xnT[:], xnT_ps[:])
                G = []
                for half in range(2):
                    h1 = pp_big.tile([128, F], F32, tag="big_ps", name="h1")
                    nc.tensor.matmul(h1[:], w1p_sb[:, l, half, :], xnT[:],
                                     start=True, stop=True,
                                     skip_group_check=True)
                    gh = big.tile([128, F], BF16, tag=f"g{half}", name="gh")
                    nc.scalar.activation(
                        gh[:], h1[:], AF.Gelu,
                        bias=b1p_sb[:, 2 * l + half:2 * l + half + 1])
                    G.append(gh)
                vT_ps = pp_big.tile([128, F], F32, tag="big_ps", name="vT_ps")
                for kh in range(2):
                    nc.tensor.matmul(vT_ps[:], w2p_sb[:, l, kh, :], G[kh][:],
                                     start=(kh == 0), stop=(kh == 1),
                                     skip_group_check=True)
                vtT = big.tile([128, F], F32, tag="vtT", name="vtT")
                nc.scalar.copy(vtT[:], vT_ps[:])
                for blk in range(NBLK):
                    nc.tensor.matmul(partial_ps[:, blk, :],
                                     vtT[:, blk * 128:(blk + 1) * 128],
                                     identf[:], is_transpose=True,
                                     start=(j == 0 and blk == 0),
                                     stop=(j == GROUP - 1 or l == L - 2),
                                     skip_group_check=True)
                nc.vector.tensor_copy(slots[:, g + 1, :, :], partial_ps[:])
                if j == GROUP - 1:
                    creation_pre(st, slots[:, g + 1, :, :], g + 1, sh)
            if j == GROUP - 1:
                creation_finish(sts, g + 1, sh)

        NS = 4 if tiles_per_core % 4 == 0 else (
            2 if tiles_per_core % 2 == 0 else 1)
        spd = es.enter_context(tc.tile_pool(name="spd", bufs=1))
        with tc.For_i(0, tiles_per_core // NS, 1) as it0:
            sh = {}
            sh["sdots_all"] = spd.tile([128, NS, 5, NBLK, L], F32,
                                       tag="sdots_all", name="sdots_all")
            sh["stats_sb_all"] = spd.tile([128, NS, NBLK, 33], F32,
                                          tag="stats_sb_all", name="stats_all")
            sts = [tile_start(it0 * NS + k, k, sh) for k in range(NS)]
            for st in sts:
                creation_pre(st, st["slots"][:, 0, :, :], 0, sh)
            creation_finish(sts, 0, sh)
            for l in range(L):
                emit_layer_quad(sts, l, sh)

    nc.finalize()
    return nc


def _prep_consts(w, ln_g, ln_b, W1, b1, W2):
    bf = ml_dtypes.bfloat16
    W1p = ln_g[:, :, None] * W1                                   # diag(g) @ W1
    b1p = b1 + np.einsum("ld,ldm->lm", ln_b, W1)                  # b1 + ln_b @ W1
    w1p = np.ascontiguousarray(W1p.transpose(1, 0, 2)).reshape(D, L * 2 * 128)
    b1p_sb = b1p.reshape(L, 2, 128).transpose(2, 0, 1).reshape(128, 2 * L)
    w2p = W2.reshape(L, 2, 128, D).transpose(2, 0, 1, 3)
    w2p = np.ascontiguousarray(w2p).reshape(128, L * 2 * D)
    return {
        "w_t": np.ascontiguousarray(w.T).astype(np.float32),
        "w_f": np.ascontiguousarray(w).astype(np.float32),
        "w1p": w1p.astype(bf),
        "b1p": np.ascontiguousarray(b1p_sb).astype(np.float32),
        "w2p": w2p.astype(bf),
    }


def kernel(embedding, w, ln_g, ln_b, W1, b1, W2, b2, _tiles=16, _trace=False, _dbg=False):
    if _trace:
        _install_ntff_hook()
    B, T, Dd = embedding.shape
    assert Dd == D
    n_tok = _tiles * F

    key = ("k", _tiles, bool(_dbg))
    if key not in _CACHE:
        _CACHE[key] = build(_tiles, dbg=_dbg)
    nc = _CACHE[key]

    assert np.all(np.asarray(b2) == 0.0), "nonzero b2 unsupported"
    consts = _prep_consts(np.asarray(w, np.float32),
                          np.asarray(ln_g, np.float32),
                          np.asarray(ln_b, np.float32),
                          np.asarray(W1, np.float32),
                          np.asarray(b1, np.float32),
                          np.asarray(W2, np.float32))
    emb_full = np.asarray(embedding, np.float32).reshape(B * T, D)

    per_core = B * T // N_CORES
    in_maps = []
    for c in range(N_CORES):
        shard = emb_full[c * per_core:(c + 1) * per_core][:n_tok]
        in_maps.append({"emb": np.ascontiguousarray(shard), **consts})

    res = run_bass_kernel_spmd(nc, in_maps, core_ids=list(range(N_CORES)),
                               trace=_trace)
    outs = [res.results[c]["out"] for c in range(N_CORES)]
    if _dbg:
        kernel.dbg = {k: res.results[0][k] for k in ("dbgH", "dbgW", "dbgS")}
    full = np.stack(outs).reshape(N_CORES, n_tok, D)
    kernel.last_exec_ns = getattr(res, "exec_time_ns", None)
    kernel.last_mean_ns = getattr(res, "mean_exec_time_ns", None)
    if n_tok == per_core:
        return full.reshape(B, T, D)
    return full  # debug partial run


# revision 23
# speedup vs baseline: 2.0258x; 2.0258x over previous
"""Trainium2 Bass kernel for nn_BlockAttnRes (block-softmax residual net).

Shapes: embedding [8, 8192, 128] f32, L=16 layers, BLOCK_SIZE=4.
Sharding: batch dim B=8 across 8 cores (1 batch row / core = 8192 tokens).

Per-core: tokens-on-partitions ("row") bf16 state resident in SBUF.
6 state slots: slot0 = emb, slot 1+g = partial of group g (becomes block g+1
at commit). Loop: For_i over token tiles (F=512 tokens = 4 blocks of 128),
python-unrolled 16 layers inside. Key techniques:
  - static-source logits for all layers precomputed at source creation
    (PE f32r matmuls + tiny chunk transposes)
  - partial-source stats per layer via fused scalar_tensor_tensor accums
  - softmax-exp via tanh identity e^t=(1+T)/(1-T) (gelu ACT table set only)
  - rsqrt via int bit-trick seed + 2 Newton iterations (DVE only)
  - weighted sums via per-block fused stt chains
  - LayerNorm affine folded into W1' = diag(g)@W1, b1' = b1 + ln_b@W1 (host)
  - MLP: PE transposes to col layout, bf16 matmuls, ACT gelu fused bias
  - partial accumulated in PSUM by f32 transpose-matmuls (start=False accum)
"""
import contextlib
import ctypes
import sys
import types
from contextlib import ExitStack

sys.path.insert(0, "/opt/trn_rl_repo")


def _install_ntff_hook():
    """Provide antenv.axon_hooks (missing in the trimmed repo) so
    run_bass_kernel_spmd(trace=True) can collect NTFF profiles."""
    if "antenv.axon_hooks" in sys.modules:
        return
    try:
        lib = ctypes.CDLL("/opt/axon/libaxon_pjrt.so")
    except OSError:
        return
    if not hasattr(lib, "axon_start_nrt_profile"):
        hook = None
    else:
        lib.axon_start_nrt_profile.argtypes = [
            ctypes.POINTER(ctypes.c_int64), ctypes.c_size_t]
        lib.axon_start_nrt_profile.restype = ctypes.c_int64
        lib.axon_stop_nrt_profile.argtypes = [ctypes.c_char_p]
        lib.axon_stop_nrt_profile.restype = ctypes.c_int64

        @contextlib.contextmanager
        def hook(output_dir, device_ids):
            import jax
            jax.devices()
            if device_ids:
                ids = (ctypes.c_int64 * len(device_ids))(*device_ids)
                rc = lib.axon_start_nrt_profile(ids, len(device_ids))
            else:
                rc = lib.axon_start_nrt_profile(None, 0)
            if rc != 0:
                raise RuntimeError(f"axon_start_nrt_profile rc={rc}")
            try:
                yield
            finally:
                n = lib.axon_stop_nrt_profile(str(output_dir).encode())
                print(f"profile: {n} file(s) -> {output_dir}", file=sys.stderr)

    mod = types.ModuleType("antenv.axon_hooks")
    mod.get_axon_ntff_profile_hook = lambda: hook
    mod.set_axon_ntff_profile_hook = lambda h: None
    sys.modules["antenv.axon_hooks"] = mod

import numpy as np
import ml_dtypes

import concourse.bacc as bacc
import concourse.bass as bass
import concourse.mybir as mybir
from concourse.bass_utils import run_bass_kernel_spmd
from concourse.tile import TileContext
from concourse.masks import make_identity

F32 = mybir.dt.float32
BF16 = mybir.dt.bfloat16
F32R = mybir.dt.float32r
I32 = mybir.dt.int32
ALU = mybir.AluOpType
AF = mybir.ActivationFunctionType
AX = mybir.AxisListType

L = 16
GROUP = 4
D = 128
NBLK = 4                 # 128-token blocks per tile
F = NBLK * 128           # tokens per tile
EPS_RMS = 1e-8
EPS_LN = 1e-5
MAGIC = 0x5F3759DF
N_CORES = 8

_CACHE = {}


def _bcast(ap, n):
    """Append a stride-0 inner free dim of size n to an AP."""
    return bass.AP(tensor=ap.tensor, offset=ap.offset,
                   ap=list(ap.ap) + [[0, n]])


def _newton_rsqrt(nc, pool, x, shape, iters=2):
    """y = rsqrt(x) for x [128, *shape] f32 tile (positive). Returns y tile."""
    y = pool.tile([128] + list(shape), F32, tag="nw_y", name="nw_y")
    xi = x.bitcast(I32)
    yi = y.bitcast(I32)
    nc.vector.tensor_scalar(out=yi[:], in0=xi[:], scalar1=1, scalar2=0,
                            op0=ALU.logical_shift_right,
                            op1=ALU.logical_shift_right)
    nc.vector.tensor_scalar(out=yi[:], in0=yi[:], scalar1=-1, scalar2=MAGIC,
                            op0=ALU.mult, op1=ALU.add)
    t = pool.tile([128] + list(shape), F32, tag="nw_t", name="nw_t")
    for _ in range(iters):
        nc.vector.tensor_mul(t[:], y[:], y[:])
        nc.vector.scalar_tensor_tensor(out=t[:], in0=t[:], scalar=-0.5,
                                       in1=x[:], op0=ALU.mult, op1=ALU.mult)
        nc.vector.scalar_tensor_tensor(out=y[:], in0=t[:], scalar=1.5,
                                       in1=y[:], op0=ALU.add, op1=ALU.mult)
    return y


def build(tiles_per_core=16, dbg=False):
    nc = bacc.Bacc("TRN2", target_bir_lowering=False)
    n_tok = tiles_per_core * F

    emb = nc.dram_tensor("emb", [n_tok, D], F32, kind="ExternalInput")
    w_t = nc.dram_tensor("w_t", [D, L], F32, kind="ExternalInput")
    w_f = nc.dram_tensor("w_f", [L, D], F32, kind="ExternalInput")
    w1p = nc.dram_tensor("w1p", [D, L * 2 * 128], BF16, kind="ExternalInput")
    b1p = nc.dram_tensor("b1p", [128, 2 * L], F32, kind="ExternalInput")
    w2p = nc.dram_tensor("w2p", [128, L * 2 * D], BF16, kind="ExternalInput")
    out = nc.dram_tensor("out", [n_tok, D], F32, kind="ExternalOutput")
    if dbg:
        dbgH = nc.dram_tensor("dbgH", [L, 128, NBLK, D], BF16, kind="ExternalOutput")
        dbgW = nc.dram_tensor("dbgW", [L, 128, NBLK, 5], F32, kind="ExternalOutput")
        dbgS = nc.dram_tensor("dbgS", [128, 5, NBLK, L], F32, kind="ExternalOutput")

    emb_v = emb.rearrange("(T b p) d -> T p b d", b=NBLK, p=128)
    out_v = out.rearrange("(T b p) d -> T p b d", b=NBLK, p=128)

    with TileContext(nc) as tc, ExitStack() as es:
        cst = es.enter_context(tc.tile_pool(name="cst", bufs=1))
        ident16 = cst.tile([128, 128], BF16)
        make_identity(nc, ident16[:])
        identf = cst.tile([128, 128], F32)
        make_identity(nc, identf[:])
        identr = cst.tile([128, 128], F32R)
        nc.vector.tensor_copy(identr[:], identf[:])

        wallT = cst.tile([128, L], F32)
        nc.sync.dma_start(wallT[:], w_t[:])
        wallT_r = cst.tile([128, L], F32R)
        nc.vector.tensor_copy(wallT_r[:], wallT[:])
        onesf = cst.tile([128, 1], F32)
        nc.vector.memset(onesf[:], 1.0)
        ones_r = cst.tile([128, 1], F32R)
        nc.vector.tensor_copy(ones_r[:], onesf[:])

        # w broadcast across partitions: [128, L, D] f32
        wrep = cst.tile([128, L, D], F32)
        wfa = w_f[:]
        nc.gpsimd.dma_start(
            out=wrep[:],
            in_=bass.AP(tensor=wfa.tensor, offset=wfa.offset,
                        ap=[[0, 128]] + list(wfa.ap)))

        w1p_sb = cst.tile([128, L, 2, 128], BF16)
        nc.sync.dma_start(w1p_sb[:], w1p[:].rearrange(
            "d (l h m) -> d l h m", l=L, h=2))
        b1p_sb = cst.tile([128, 2 * L], F32)
        nc.sync.dma_start(b1p_sb[:], b1p[:])
        w2p_sb = cst.tile([128, L, 2, D], BF16)
        nc.sync.dma_start(w2p_sb[:], w2p[:].rearrange(
            "m (l k d) -> m l k d", l=L, k=2))

        zero16 = cst.tile([128, NBLK, D], BF16)
        nc.vector.memset(zero16[:], 0.0)

        sp = es.enter_context(tc.tile_pool(name="state", bufs=4))
        big = es.enter_context(tc.tile_pool(name="big", bufs=4))
        sml = es.enter_context(tc.tile_pool(name="sml", bufs=12))
        nwp = es.enter_context(tc.tile_pool(name="nw", bufs=12))
        pp_xnT = es.enter_context(tc.tile_pool(name="pp_xnT", bufs=1, space="PSUM"))
        pp_big = es.enter_context(tc.tile_pool(name="pp_big", bufs=3, space="PSUM"))
        pp_par = es.enter_context(tc.tile_pool(name="pp_par", bufs=4, space="PSUM"))

        def creation_stats(slot_buf, sdots, s_idx):
            srcT_ps = pp_xnT.tile([128, F], F32R, tag="xnT_ps", name="srcT_ps")
            for blk in range(NBLK):
                nc.tensor.matmul(srcT_ps[:, blk * 128:(blk + 1) * 128],
                                 slot_buf[:, blk, :], identr[:],
                                 is_transpose=True, start=True, stop=True,
                                 skip_group_check=True)
            srcT_r = big.tile([128, F], F32R, tag="srcT_r")
            nc.vector.tensor_copy(srcT_r[:], srcT_ps[:])
            sq_r = big.tile([128, F], F32R, tag="sq_r")
            nc.scalar.activation(sq_r[:], srcT_r[:], AF.Square)
            dots_ps = pp_big.tile([33, F], F32, tag="big_ps")
            nc.tensor.matmul(dots_ps[0:16, :], wallT_r[:], srcT_r[:],
                             start=True, stop=True, skip_group_check=True)
            ms_ps = pp_big.tile([1, F], F32, tag="big_ps")
            nc.tensor.matmul(ms_ps[:], ones_r[:], sq_r[:],
                             start=True, stop=True, skip_group_check=True)
            dots_sb = big.tile([33, F], F32, tag="dots_sb")
            nc.vector.tensor_copy(dots_sb[0:16, :], dots_ps[0:16, :])
            nc.vector.tensor_copy(dots_sb[32:33, :], ms_ps[:])
            statT_ps = pp_big.tile([128, NBLK, 33], F32, tag="big_ps")
            for c in range(NBLK):
                nc.tensor.matmul(statT_ps[:, c, :],
                                 dots_sb[:, c * 128:(c + 1) * 128],
                                 identf[0:33, 0:33],
                                 is_transpose=True, start=True, stop=True,
                                 skip_group_check=True)
            stats_sb = sml.tile([128, NBLK, 33], F32, tag="stats_sb")
            nc.vector.tensor_copy(stats_sb[:], statT_ps[:])
            xs = sml.tile([128, NBLK], F32, tag="xs_cr")
            nc.vector.tensor_scalar(out=xs[:], in0=stats_sb[:, :, 32],
                                    scalar1=1.0 / D, scalar2=EPS_RMS,
                                    op0=ALU.mult, op1=ALU.add)
            rms = _newton_rsqrt(nc, nwp, xs, xs.shape[1:])
            nc.vector.scalar_tensor_tensor(
                out=sdots[:, s_idx, :, :], in0=stats_sb[:, :, 0:16],
                scalar=1.0, in1=_bcast(rms[:], L),
                op0=ALU.bypass, op1=ALU.mult)

        def tile_start(it, k, sh):
            st = {"it": it, "k": k, "sh": sh}
            st["slots"] = sp.tile([128, 6, NBLK, D], F32R, tag="slots", name="slots")
            st["trash"] = sp.tile([128, NBLK, D], BF16, tag="trash", name="trash")
            st["sdots"] = sh["sdots_all"][:, k]
            emb_st = big.tile([128, NBLK, D], F32, tag="emb_st")
            nc.sync.dma_start(emb_st[:], emb_v[bass.ds(it, 1)])
            nc.vector.tensor_copy(st["slots"][:, 0, :, :], emb_st[:])
            st["partial_ps"] = pp_par.tile([128, NBLK, D], F32, tag="par", name="par")
            return st

        def creation_pre(st, slot_buf, s_idx, sh):
            """Per-stream creation: transposes + stat matmuls -> stats_sb_all."""
            k = st["k"]
            srcT_ps = pp_xnT.tile([128, F], F32R, tag="xnT_ps", name="srcT_ps")
            for blk in range(NBLK):
                nc.tensor.matmul(srcT_ps[:, blk * 128:(blk + 1) * 128],
                                 slot_buf[:, blk, :], identr[:],
                                 is_transpose=True, start=True, stop=True,
                                 skip_group_check=True)
            srcT_r = big.tile([128, F], F32R, tag="srcT_r")
            nc.vector.tensor_copy(srcT_r[:], srcT_ps[:])
            sq_r = big.tile([128, F], F32R, tag="sq_r")
            nc.scalar.activation(sq_r[:], srcT_r[:], AF.Square)
            dots_ps = pp_big.tile([33, F], F32, tag="big_ps", name="dots_ps")
            nc.tensor.matmul(dots_ps[0:16, :], wallT_r[:], srcT_r[:],
                             start=True, stop=True, skip_group_check=True)
            ms_ps = pp_big.tile([1, F], F32, tag="big_ps", name="ms_ps")
            nc.tensor.matmul(ms_ps[:], ones_r[:], sq_r[:],
                             start=True, stop=True, skip_group_check=True)
            dots_sb = big.tile([33, F], F32, tag="dots_sb")
            nc.vector.tensor_copy(dots_sb[0:16, :], dots_ps[0:16, :])
            nc.vector.tensor_copy(dots_sb[32:33, :], ms_ps[:])
            statT_ps = pp_big.tile([128, NBLK, 33], F32, tag="big_ps",
                                   name="statT_ps")
            for c in range(NBLK):
                nc.tensor.matmul(statT_ps[:, c, :],
                                 dots_sb[:, c * 128:(c + 1) * 128],
                                 identf[0:33, 0:33],
                                 is_transpose=True, start=True, stop=True,
                                 skip_group_check=True)
            nc.vector.tensor_copy(sh["stats_sb_all"][:, k], statT_ps[:])

        def creation_finish(sts, s_idx, sh):
            """Batched across streams: rms + scaled dots."""
            ns_ = len(sts)
            stats = sh["stats_sb_all"]
            xs = sml.tile([128, NS, NBLK], F32, tag="xs_cr")
            nc.vector.tensor_scalar(out=xs[:, 0:ns_], in0=stats[:, 0:ns_, :, 32],
                                    scalar1=1.0 / D, scalar2=EPS_RMS,
                                    op0=ALU.mult, op1=ALU.add)
            rms = _newton_rsqrt(nc, nwp, xs, (NS, NBLK))
            r_ap = rms[:, 0:ns_]
            nc.vector.scalar_tensor_tensor(
                out=sh["sdots_all"][:, 0:ns_, s_idx, :, :],
                in0=stats[:, 0:ns_, :, 0:16],
                scalar=1.0, in1=_bcast(r_ap, L),
                op0=ALU.bypass, op1=ALU.mult)

        def emit_layer_quad(sts, l, sh):
            ns_ = len(sts)
            g, j = l // GROUP, l % GROUP
            nsrc = g + 1
            has_p = j > 0
            n = nsrc + (1 if has_p else 0)
            sdots_all = sh["sdots_all"]

            E_T = sml.tile([128, NS, NBLK, 5], F32, tag="E_T")
            e_ap = E_T[:]
            statics_out = bass.AP(
                tensor=e_ap.tensor, offset=e_ap.offset,
                ap=[e_ap.ap[0], [NBLK * 5, ns_], [1, nsrc], [5, NBLK]])
            nc.scalar.activation(out=statics_out,
                                 in_=sdots_all[:, 0:ns_, 0:nsrc, :, l],
                                 func=AF.Tanh, scale=0.5)

            if has_p:
                dotp = sml.tile([128, NS, NBLK], F32, tag="dotp")
                ssqp = sml.tile([128, NS, NBLK], F32, tag="ssqp")
                for st in sts:
                    k = st["k"]
                    p_ps = st["slots"][:, g + 1, :, :]
                    for blk in range(NBLK):
                        nc.vector.scalar_tensor_tensor(
                            out=st["trash"][:, blk, :], in0=p_ps[:, blk, :],
                            scalar=1.0, in1=wrep[:, l, :],
                            op0=ALU.bypass, op1=ALU.mult,
                            accum_out=dotp[:, k, blk:blk + 1])
                        nc.scalar.activation(
                            out=st["trash"][:, blk, :], in_=p_ps[:, blk, :],
                            func=AF.Square,
                            accum_out=ssqp[:, k, blk:blk + 1])
                xp = sml.tile([128, NS, NBLK], F32, tag="xp")
                nc.vector.tensor_scalar(out=xp[:, 0:ns_], in0=ssqp[:, 0:ns_],
                                        scalar1=1.0 / D, scalar2=EPS_RMS,
                                        op0=ALU.mult, op1=ALU.add)
                rmsp = _newton_rsqrt(nc, nwp, xp, (NS, NBLK), iters=1)
                lp = sml.tile([128, NS, NBLK], F32, tag="lp")
                nc.vector.tensor_mul(lp[:, 0:ns_], dotp[:, 0:ns_],
                                     rmsp[:, 0:ns_])
                nc.scalar.activation(out=E_T[:, 0:ns_, :, nsrc],
                                     in_=lp[:, 0:ns_],
                                     func=AF.Tanh, scale=0.5)

            Ev = E_T[:, 0:ns_, :, 0:n]
            A = sml.tile([128, NS, NBLK, 5], F32, tag="A")
            nc.vector.tensor_scalar_add(A[:, 0:ns_, :, 0:n], Ev, 1.0)
            Bt = sml.tile([128, NS, NBLK, 5], F32, tag="B")
            nc.vector.tensor_scalar(out=Bt[:, 0:ns_, :, 0:n], in0=Ev,
                                    scalar1=-1.0, scalar2=-1.0,
                                    op0=ALU.mult, op1=ALU.subtract)
            R = sml.tile([128, NS, NBLK, 5], F32, tag="R")
            nc.vector.reciprocal(R[:, 0:ns_, :, 0:n], Bt[:, 0:ns_, :, 0:n])
            E = sml.tile([128, NS, NBLK, 5], F32, tag="E")
            nc.vector.tensor_mul(E[:, 0:ns_, :, 0:n], A[:, 0:ns_, :, 0:n],
                                 R[:, 0:ns_, :, 0:n])
            den = sml.tile([128, NS, NBLK], F32, tag="den")
            nc.vector.tensor_reduce(den[:, 0:ns_], E[:, 0:ns_, :, 0:n],
                                    axis=AX.X, op=ALU.add)
            rd = sml.tile([128, NS, NBLK], F32, tag="rd")
            nc.vector.reciprocal(rd[:, 0:ns_], den[:, 0:ns_])
            wts = sml.tile([128, NS, NBLK, 5], F32, tag="wts")
            rd_ap = rd[:, 0:ns_]
            nc.vector.scalar_tensor_tensor(
                out=wts[:, 0:ns_, :, 0:n], in0=E[:, 0:ns_, :, 0:n], scalar=1.0,
                in1=_bcast(rd_ap, n), op0=ALU.bypass, op1=ALU.mult)

            last = l == L - 1
            hsum = sml.tile([128, NS, NBLK], F32, tag="hsum")
            hs = []
            for st in sts:
                k = st["k"]
                slots, partial_ps = st["slots"], st["partial_ps"]

                def wsrc(i, blk):
                    return slots[:, i, blk, :]

                h = big.tile([128, NBLK, D], F32 if last else F32R,
                             tag="h_f32" if last else "h", name="h")
                hs.append(h)
                for blk in range(NBLK):
                    if n == 1:
                        nc.vector.tensor_scalar(
                            out=h[:, blk, :], in0=wsrc(0, blk),
                            scalar1=wts[:, k, blk, 0:1], scalar2=0.0,
                            op0=ALU.mult, op1=ALU.add,
                            accum_out=hsum[:, k, blk:blk + 1])
                    else:
                        nc.vector.tensor_scalar(
                            out=h[:, blk, :], in0=wsrc(0, blk),
                            scalar1=wts[:, k, blk, 0:1], scalar2=None,
                            op0=ALU.mult)
                    for i in range(1, n):
                        nc.vector.scalar_tensor_tensor(
                            out=h[:, blk, :], in0=wsrc(i, blk),
                            scalar=wts[:, k, blk, i:i + 1],
                            in1=h[:, blk, :],
                            op0=ALU.mult, op1=ALU.add,
                            accum_out=(hsum[:, k, blk:blk + 1]
                                       if i == n - 1 else None))
                if dbg and k == 0:
                    nc.sync.dma_start(dbgW[l], wts[:, 0])
                    if not last:
                        nc.sync.dma_start(dbgH[l], h[:])
                if last:
                    nc.sync.dma_start(out_v[bass.ds(st["it"], 1)], h[:])
                    if dbg and k == 0:
                        nc.sync.dma_start(dbgS[:], st["sdots"])
            if last:
                return

            hssq = sml.tile([128, NS, NBLK], F32, tag="hssq")
            for st, h in zip(sts, hs):
                k = st["k"]
                for blk in range(NBLK):
                    nc.scalar.activation(
                        out=st["trash"][:, blk, :], in_=h[:, blk, :],
                        func=AF.Square,
                        accum_out=hssq[:, k, blk:blk + 1])
            m2 = sml.tile([128, NS, NBLK], F32, tag="m2")
            nc.vector.tensor_mul(m2[:, 0:ns_], hsum[:, 0:ns_], hsum[:, 0:ns_])
            t1 = sml.tile([128, NS, NBLK], F32, tag="t1")
            nc.vector.tensor_scalar(out=t1[:, 0:ns_], in0=hssq[:, 0:ns_],
                                    scalar1=1.0 / D, scalar2=EPS_LN,
                                    op0=ALU.mult, op1=ALU.add)
            xs2 = sml.tile([128, NS, NBLK], F32, tag="xs2")
            nc.vector.scalar_tensor_tensor(
                out=xs2[:, 0:ns_], in0=m2[:, 0:ns_], scalar=-1.0 / (D * D),
                in1=t1[:, 0:ns_], op0=ALU.mult, op1=ALU.add)
            s_ln = _newton_rsqrt(nc, nwp, xs2, (NS, NBLK), iters=1)
            mu = sml.tile([128, NS, NBLK], F32, tag="mu")
            nc.vector.tensor_scalar_mul(mu[:, 0:ns_], hsum[:, 0:ns_], 1.0 / D)

            for st, h in zip(sts, hs):
                k = st["k"]
                slots, partial_ps = st["slots"], st["partial_ps"]
                xn = big.tile([128, NBLK, D], BF16, tag="xn", name="xn")
                for blk in range(NBLK):
                    nc.vector.tensor_scalar(
                        out=xn[:, blk, :], in0=h[:, blk, :],
                        scalar1=mu[:, k, blk:blk + 1],
                        scalar2=s_ln[:, k, blk:blk + 1],
                        op0=ALU.subtract, op1=ALU.mult)
                xnT_ps = pp_xnT.tile([128, F], BF16, tag="xnT_ps", name="xnT_ps")
                for blk in range(NBLK):
                    nc.tensor.matmul(xnT_ps[:, blk * 128:(blk + 1) * 128],
                                     xn[:, blk, :], ident16[:],
                                     is_transpose=True, start=True, stop=True,
                                     skip_group_check=True)
                xnT = big.tile([128, F], BF16, tag="xnT", name="xnT")
                nc.scalar.copy(xnT[:], xnT_ps[:])
                G = []
                for half in range(2):
                    h1 = pp_big.tile([128, F], F32, tag="big_ps", name="h1")
                    nc.tensor.matmul(h1[:], w1p_sb[:, l, half, :], xnT[:],
                                     start=True, stop=True,
                                     skip_group_check=True)
                    gh = big.tile([128, F], BF16, tag=f"g{half}", name="gh")
                    nc.scalar.activation(
                        gh[:], h1[:], AF.Gelu,
                        bias=b1p_sb[:, 2 * l + half:2 * l + half + 1])
                    G.append(gh)
                vT_ps = pp_big.tile([128, F], F32, tag="big_ps", name="vT_ps")
                for kh in range(2):
                    nc.tensor.matmul(vT_ps[:], w2p_sb[:, l, kh, :], G[kh][:],
                                     start=(kh == 0), stop=(kh == 1),
                                     skip_group_check=True)
                vtT = big.tile([128, F], F32, tag="vtT", name="vtT")
                nc.vector.tensor_copy(vtT[:], vT_ps[:])
                for blk in range(NBLK):
                    nc.tensor.matmul(partial_ps[:, blk, :],
                                     vtT[:, blk * 128:(blk + 1) * 128],
                                     identf[:], is_transpose=True,
                                     start=(j == 0 and blk == 0),
                                     stop=(j == GROUP - 1 or l == L - 2),
                                     skip_group_check=True)
                nc.vector.tensor_copy(slots[:, g + 1, :, :], partial_ps[:])
                if j == GROUP - 1:
                    creation_pre(st, slots[:, g + 1, :, :], g + 1, sh)
            if j == GROUP - 1:
                creation_finish(sts, g + 1, sh)

        NS = 4 if tiles_per_core % 4 == 0 else (
            2 if tiles_per_core % 2 == 0 else 1)
        spd = es.enter_context(tc.tile_pool(name="spd", bufs=1))
        with tc.For_i(0, tiles_per_core // NS, 1) as it0:
            sh = {}
            sh["sdots_all"] = spd.tile([128, NS, 5, NBLK, L], F32,
                                       tag="sdots_all", name="sdots_all")
            sh["stats_sb_all"] = spd.tile([128, NS, NBLK, 33], F32,
                                          tag="stats_sb_all", name="stats_all")
            sts = [tile_start(it0 * NS + k, k, sh) for k in range(NS)]
            for st in sts:
                creation_pre(st, st["slots"][:, 0, :, :], 0, sh)
            creation_finish(sts, 0, sh)
            for l in range(L):
                emit_layer_quad(sts, l, sh)

    nc.finalize()
    return nc


def _prep_consts(w, ln_g, ln_b, W1, b1, W2):
    bf = ml_dtypes.bfloat16
    W1p = ln_g[:, :, None] * W1                                   # diag(g) @ W1
    b1p = b1 + np.einsum("ld,ldm->lm", ln_b, W1)                  # b1 + ln_b @ W1
    w1p = np.ascontiguousarray(W1p.transpose(1, 0, 2)).reshape(D, L * 2 * 128)
    b1p_sb = b1p.reshape(L, 2, 128).transpose(2, 0, 1).reshape(128, 2 * L)
    w2p = W2.reshape(L, 2, 128, D).transpose(2, 0, 1, 3)
    w2p = np.ascontiguousarray(w2p).reshape(128, L * 2 * D)
    return {
        "w_t": np.ascontiguousarray(w.T).astype(np.float32),
        "w_f": np.ascontiguousarray(w).astype(np.float32),
        "w1p": w1p.astype(bf),
        "b1p": np.ascontiguousarray(b1p_sb).astype(np.float32),
        "w2p": w2p.astype(bf),
    }


def kernel(embedding, w, ln_g, ln_b, W1, b1, W2, b2, _tiles=16, _trace=False, _dbg=False):
    if _trace:
        _install_ntff_hook()
    B, T, Dd = embedding.shape
    assert Dd == D
    n_tok = _tiles * F

    key = ("k", _tiles, bool(_dbg))
    if key not in _CACHE:
        _CACHE[key] = build(_tiles, dbg=_dbg)
    nc = _CACHE[key]

    assert np.all(np.asarray(b2) == 0.0), "nonzero b2 unsupported"
    consts = _prep_consts(np.asarray(w, np.float32),
                          np.asarray(ln_g, np.float32),
                          np.asarray(ln_b, np.float32),
                          np.asarray(W1, np.float32),
                          np.asarray(b1, np.float32),
                          np.asarray(W2, np.float32))
    emb_full = np.asarray(embedding, np.float32).reshape(B * T, D)

    per_core = B * T // N_CORES
    in_maps = []
    for c in range(N_CORES):
        shard = emb_full[c * per_core:(c + 1) * per_core][:n_tok]
        in_maps.append({"emb": np.ascontiguousarray(shard), **consts})

    res = run_bass_kernel_spmd(nc, in_maps, core_ids=list(range(N_CORES)),
                               trace=_trace)
    outs = [res.results[c]["out"] for c in range(N_CORES)]
    if _dbg:
        kernel.dbg = {k: res.results[0][k] for k in ("dbgH", "dbgW", "dbgS")}
    full = np.stack(outs).reshape(N_CORES, n_tok, D)
    kernel.last_exec_ns = getattr(res, "exec_time_ns", None)
    kernel.last_mean_ns = getattr(res, "mean_exec_time_ns", None)
    if n_tok == per_core:
        return full.reshape(B, T, D)
    return full  # debug partial run


# revision 24
# speedup vs baseline: 2.0386x; 1.0063x over previous
"""Trainium2 Bass kernel for nn_BlockAttnRes (block-softmax residual net).

Shapes: embedding [8, 8192, 128] f32, L=16 layers, BLOCK_SIZE=4.
Sharding: batch dim B=8 across 8 cores (1 batch row / core = 8192 tokens).

Per-core: tokens-on-partitions ("row") bf16 state resident in SBUF.
6 state slots: slot0 = emb, slot 1+g = partial of group g (becomes block g+1
at commit). Loop: For_i over token tiles (F=512 tokens = 4 blocks of 128),
python-unrolled 16 layers inside. Key techniques:
  - static-source logits for all layers precomputed at source creation
    (PE f32r matmuls + tiny chunk transposes)
  - partial-source stats per layer via fused scalar_tensor_tensor accums
  - softmax-exp via tanh identity e^t=(1+T)/(1-T) (gelu ACT table set only)
  - rsqrt via int bit-trick seed + 2 Newton iterations (DVE only)
  - weighted sums via per-block fused stt chains
  - LayerNorm affine folded into W1' = diag(g)@W1, b1' = b1 + ln_b@W1 (host)
  - MLP: PE transposes to col layout, bf16 matmuls, ACT gelu fused bias
  - partial accumulated in PSUM by f32 transpose-matmuls (start=False accum)
"""
import contextlib
import ctypes
import sys
import types
from contextlib import ExitStack

sys.path.insert(0, "/opt/trn_rl_repo")


def _install_ntff_hook():
    """Provide antenv.axon_hooks (missing in the trimmed repo) so
    run_bass_kernel_spmd(trace=True) can collect NTFF profiles."""
    if "antenv.axon_hooks" in sys.modules:
        return
    try:
        lib = ctypes.CDLL("/opt/axon/libaxon_pjrt.so")
    except OSError:
        return
    if not hasattr(lib, "axon_start_nrt_profile"):
        hook = None
    else:
        lib.axon_start_nrt_profile.argtypes = [
            ctypes.POINTER(ctypes.c_int64), ctypes.c_size_t]
        lib.axon_start_nrt_profile.restype = ctypes.c_int64
        lib.axon_stop_nrt_profile.argtypes = [ctypes.c_char_p]
        lib.axon_stop_nrt_profile.restype = ctypes.c_int64

        @contextlib.contextmanager
        def hook(output_dir, device_ids):
            import jax
            jax.devices()
            if device_ids:
                ids = (ctypes.c_int64 * len(device_ids))(*device_ids)
                rc = lib.axon_start_nrt_profile(ids, len(device_ids))
            else:
                rc = lib.axon_start_nrt_profile(None, 0)
            if rc != 0:
                raise RuntimeError(f"axon_start_nrt_profile rc={rc}")
            try:
                yield
            finally:
                n = lib.axon_stop_nrt_profile(str(output_dir).encode())
                print(f"profile: {n} file(s) -> {output_dir}", file=sys.stderr)

    mod = types.ModuleType("antenv.axon_hooks")
    mod.get_axon_ntff_profile_hook = lambda: hook
    mod.set_axon_ntff_profile_hook = lambda h: None
    sys.modules["antenv.axon_hooks"] = mod

import numpy as np
import ml_dtypes

import concourse.bacc as bacc
import concourse.bass as bass
import concourse.mybir as mybir
from concourse.bass_utils import run_bass_kernel_spmd
from concourse.tile import TileContext
from concourse.masks import make_identity

F32 = mybir.dt.float32
BF16 = mybir.dt.bfloat16
F32R = mybir.dt.float32r
I32 = mybir.dt.int32
ALU = mybir.AluOpType
AF = mybir.ActivationFunctionType
AX = mybir.AxisListType

L = 16
GROUP = 4
D = 128
NBLK = 4                 # 128-token blocks per tile
F = NBLK * 128           # tokens per tile
EPS_RMS = 1e-8
EPS_LN = 1e-5
MAGIC = 0x5F3759DF
N_CORES = 8

_CACHE = {}


def _bcast(ap, n):
    """Append a stride-0 inner free dim of size n to an AP."""
    return bass.AP(tensor=ap.tensor, offset=ap.offset,
                   ap=list(ap.ap) + [[0, n]])


def _newton_rsqrt(nc, pool, x, shape, iters=2):
    """y = rsqrt(x) for x [128, *shape] f32 tile (positive). Returns y tile."""
    y = pool.tile([128] + list(shape), F32, tag="nw_y", name="nw_y")
    xi = x.bitcast(I32)
    yi = y.bitcast(I32)
    nc.vector.tensor_scalar(out=yi[:], in0=xi[:], scalar1=1, scalar2=0,
                            op0=ALU.logical_shift_right,
                            op1=ALU.logical_shift_right)
    nc.vector.tensor_scalar(out=yi[:], in0=yi[:], scalar1=-1, scalar2=MAGIC,
                            op0=ALU.mult, op1=ALU.add)
    t = pool.tile([128] + list(shape), F32, tag="nw_t", name="nw_t")
    for _ in range(iters):
        nc.vector.tensor_mul(t[:], y[:], y[:])
        nc.vector.scalar_tensor_tensor(out=t[:], in0=t[:], scalar=-0.5,
                                       in1=x[:], op0=ALU.mult, op1=ALU.mult)
        nc.vector.scalar_tensor_tensor(out=y[:], in0=t[:], scalar=1.5,
                                       in1=y[:], op0=ALU.add, op1=ALU.mult)
    return y


def build(tiles_per_core=16, dbg=False):
    nc = bacc.Bacc("TRN2", target_bir_lowering=False)
    n_tok = tiles_per_core * F

    emb = nc.dram_tensor("emb", [n_tok, D], F32, kind="ExternalInput")
    w_t = nc.dram_tensor("w_t", [D, L], F32, kind="ExternalInput")
    w_f = nc.dram_tensor("w_f", [L, D], F32, kind="ExternalInput")
    w1p = nc.dram_tensor("w1p", [D, L * 2 * 128], BF16, kind="ExternalInput")
    b1p = nc.dram_tensor("b1p", [128, 2 * L], F32, kind="ExternalInput")
    w2p = nc.dram_tensor("w2p", [128, L * 2 * D], BF16, kind="ExternalInput")
    out = nc.dram_tensor("out", [n_tok, D], F32, kind="ExternalOutput")
    if dbg:
        dbgH = nc.dram_tensor("dbgH", [L, 128, NBLK, D], BF16, kind="ExternalOutput")
        dbgW = nc.dram_tensor("dbgW", [L, 128, NBLK, 5], F32, kind="ExternalOutput")
        dbgS = nc.dram_tensor("dbgS", [128, 5, NBLK, L], F32, kind="ExternalOutput")

    emb_v = emb.rearrange("(T b p) d -> T p b d", b=NBLK, p=128)
    out_v = out.rearrange("(T b p) d -> T p b d", b=NBLK, p=128)

    with TileContext(nc) as tc, ExitStack() as es:
        cst = es.enter_context(tc.tile_pool(name="cst", bufs=1))
        ident16 = cst.tile([128, 128], BF16)
        make_identity(nc, ident16[:])
        identf = cst.tile([128, 128], F32)
        make_identity(nc, identf[:])
        identr = cst.tile([128, 128], F32R)
        nc.vector.tensor_copy(identr[:], identf[:])

        wallT = cst.tile([128, L], F32)
        nc.sync.dma_start(wallT[:], w_t[:])
        wallT_r = cst.tile([128, L], F32R)
        nc.vector.tensor_copy(wallT_r[:], wallT[:])
        onesf = cst.tile([128, 1], F32)
        nc.vector.memset(onesf[:], 1.0)
        ones_r = cst.tile([128, 1], F32R)
        nc.vector.tensor_copy(ones_r[:], onesf[:])

        # w broadcast across partitions: [128, L, D] f32
        wrep = cst.tile([128, L, D], F32)
        wfa = w_f[:]
        nc.gpsimd.dma_start(
            out=wrep[:],
            in_=bass.AP(tensor=wfa.tensor, offset=wfa.offset,
                        ap=[[0, 128]] + list(wfa.ap)))

        w1p_sb = cst.tile([128, L, 2, 128], BF16)
        nc.sync.dma_start(w1p_sb[:], w1p[:].rearrange(
            "d (l h m) -> d l h m", l=L, h=2))
        b1p_sb = cst.tile([128, 2 * L], F32)
        nc.sync.dma_start(b1p_sb[:], b1p[:])
        w2p_sb = cst.tile([128, L, 2, D], BF16)
        nc.sync.dma_start(w2p_sb[:], w2p[:].rearrange(
            "m (l k d) -> m l k d", l=L, k=2))

        zero16 = cst.tile([128, NBLK, D], BF16)
        nc.vector.memset(zero16[:], 0.0)

        sp = es.enter_context(tc.tile_pool(name="state", bufs=4))
        big = es.enter_context(tc.tile_pool(name="big", bufs=4))
        sml = es.enter_context(tc.tile_pool(name="sml", bufs=12))
        nwp = es.enter_context(tc.tile_pool(name="nw", bufs=12))
        pp_xnT = es.enter_context(tc.tile_pool(name="pp_xnT", bufs=1, space="PSUM"))
        pp_big = es.enter_context(tc.tile_pool(name="pp_big", bufs=3, space="PSUM"))
        pp_par = es.enter_context(tc.tile_pool(name="pp_par", bufs=4, space="PSUM"))

        def creation_stats(slot_buf, sdots, s_idx):
            srcT_ps = pp_xnT.tile([128, F], F32R, tag="xnT_ps", name="srcT_ps")
            for blk in range(NBLK):
                nc.tensor.matmul(srcT_ps[:, blk * 128:(blk + 1) * 128],
                                 slot_buf[:, blk, :], identr[:],
                                 is_transpose=True, start=True, stop=True,
                                 skip_group_check=True)
            srcT_r = big.tile([128, F], F32R, tag="srcT_r")
            nc.vector.tensor_copy(srcT_r[:], srcT_ps[:])
            sq_r = big.tile([128, F], F32R, tag="sq_r")
            nc.scalar.activation(sq_r[:], srcT_r[:], AF.Square)
            dots_ps = pp_big.tile([33, F], F32, tag="big_ps")
            nc.tensor.matmul(dots_ps[0:16, :], wallT_r[:], srcT_r[:],
                             start=True, stop=True, skip_group_check=True)
            ms_ps = pp_big.tile([1, F], F32, tag="big_ps")
            nc.tensor.matmul(ms_ps[:], ones_r[:], sq_r[:],
                             start=True, stop=True, skip_group_check=True)
            dots_sb = big.tile([33, F], F32, tag="dots_sb")
            nc.vector.tensor_copy(dots_sb[0:16, :], dots_ps[0:16, :])
            nc.vector.tensor_copy(dots_sb[32:33, :], ms_ps[:])
            statT_ps = pp_big.tile([128, NBLK, 33], F32, tag="big_ps")
            for c in range(NBLK):
                nc.tensor.matmul(statT_ps[:, c, :],
                                 dots_sb[:, c * 128:(c + 1) * 128],
                                 identf[0:33, 0:33],
                                 is_transpose=True, start=True, stop=True,
                                 skip_group_check=True)
            stats_sb = sml.tile([128, NBLK, 33], F32, tag="stats_sb")
            nc.vector.tensor_copy(stats_sb[:], statT_ps[:])
            xs = sml.tile([128, NBLK], F32, tag="xs_cr")
            nc.vector.tensor_scalar(out=xs[:], in0=stats_sb[:, :, 32],
                                    scalar1=1.0 / D, scalar2=EPS_RMS,
                                    op0=ALU.mult, op1=ALU.add)
            rms = _newton_rsqrt(nc, nwp, xs, xs.shape[1:])
            nc.vector.scalar_tensor_tensor(
                out=sdots[:, s_idx, :, :], in0=stats_sb[:, :, 0:16],
                scalar=1.0, in1=_bcast(rms[:], L),
                op0=ALU.bypass, op1=ALU.mult)

        def tile_start(it, k, sh):
            st = {"it": it, "k": k, "sh": sh}
            st["slots"] = sp.tile([128, 6, NBLK, D], F32R, tag="slots", name="slots")
            st["trash"] = sp.tile([128, NBLK, D], BF16, tag="trash", name="trash")
            st["sdots"] = sh["sdots_all"][:, k]
            emb_st = big.tile([128, NBLK, D], F32, tag="emb_st")
            nc.sync.dma_start(emb_st[:], emb_v[bass.ds(it, 1)])
            nc.vector.tensor_copy(st["slots"][:, 0, :, :], emb_st[:])
            st["partial_ps"] = pp_par.tile([128, NBLK, D], F32, tag="par", name="par")
            return st

        def creation_pre(st, slot_buf, s_idx, sh):
            """Per-stream creation: transposes + stat matmuls -> stats_sb_all."""
            k = st["k"]
            srcT_ps = pp_xnT.tile([128, F], F32R, tag="xnT_ps", name="srcT_ps")
            for blk in range(NBLK):
                nc.tensor.matmul(srcT_ps[:, blk * 128:(blk + 1) * 128],
                                 slot_buf[:, blk, :], identr[:],
                                 is_transpose=True, start=True, stop=True,
                                 skip_group_check=True)
            srcT_r = big.tile([128, F], F32R, tag="srcT_r")
            nc.vector.tensor_copy(srcT_r[:], srcT_ps[:])
            sq_r = big.tile([128, F], F32R, tag="sq_r")
            nc.scalar.activation(sq_r[:], srcT_r[:], AF.Square)
            dots_ps = pp_big.tile([33, F], F32, tag="big_ps", name="dots_ps")
            nc.tensor.matmul(dots_ps[0:16, :], wallT_r[:], srcT_r[:],
                             start=True, stop=True, skip_group_check=True)
            ms_ps = pp_big.tile([1, F], F32, tag="big_ps", name="ms_ps")
            nc.tensor.matmul(ms_ps[:], ones_r[:], sq_r[:],
                             start=True, stop=True, skip_group_check=True)
            dots_sb = big.tile([33, F], F32, tag="dots_sb")
            nc.vector.tensor_copy(dots_sb[0:16, :], dots_ps[0:16, :])
            nc.vector.tensor_copy(dots_sb[32:33, :], ms_ps[:])
            statT_ps = pp_big.tile([128, NBLK, 33], F32, tag="big_ps",
                                   name="statT_ps")
            for c in range(NBLK):
                nc.tensor.matmul(statT_ps[:, c, :],
                                 dots_sb[:, c * 128:(c + 1) * 128],
                                 identf[0:33, 0:33],
                                 is_transpose=True, start=True, stop=True,
                                 skip_group_check=True)
            nc.vector.tensor_copy(sh["stats_sb_all"][:, k], statT_ps[:])

        def creation_finish(sts, s_idx, sh):
            """Batched across streams: rms + scaled dots."""
            ns_ = len(sts)
            stats = sh["stats_sb_all"]
            xs = sml.tile([128, NS, NBLK], F32, tag="xs_cr")
            nc.vector.tensor_scalar(out=xs[:, 0:ns_], in0=stats[:, 0:ns_, :, 32],
                                    scalar1=1.0 / D, scalar2=EPS_RMS,
                                    op0=ALU.mult, op1=ALU.add)
            rms = _newton_rsqrt(nc, nwp, xs, (NS, NBLK))
            r_ap = rms[:, 0:ns_]
            nc.vector.scalar_tensor_tensor(
                out=sh["sdots_all"][:, 0:ns_, s_idx, :, :],
                in0=stats[:, 0:ns_, :, 0:16],
                scalar=1.0, in1=_bcast(r_ap, L),
                op0=ALU.bypass, op1=ALU.mult)

        def emit_layer_quad(sts, l, sh):
            ns_ = len(sts)
            g, j = l // GROUP, l % GROUP
            nsrc = g + 1
            has_p = j > 0
            n = nsrc + (1 if has_p else 0)
            sdots_all = sh["sdots_all"]

            E_T = sml.tile([128, NS, NBLK, 5], F32, tag="E_T")
            e_ap = E_T[:]
            statics_out = bass.AP(
                tensor=e_ap.tensor, offset=e_ap.offset,
                ap=[e_ap.ap[0], [NBLK * 5, ns_], [1, nsrc], [5, NBLK]])
            nc.scalar.activation(out=statics_out,
                                 in_=sdots_all[:, 0:ns_, 0:nsrc, :, l],
                                 func=AF.Tanh, scale=0.5)

            if has_p:
                dotp = sml.tile([128, NS, NBLK], F32, tag="dotp")
                ssqp = sml.tile([128, NS, NBLK], F32, tag="ssqp")
                for st in sts:
                    k = st["k"]
                    p_ps = st["slots"][:, g + 1, :, :]
                    for blk in range(NBLK):
                        nc.vector.scalar_tensor_tensor(
                            out=st["trash"][:, blk, :], in0=p_ps[:, blk, :],
                            scalar=1.0, in1=wrep[:, l, :],
                            op0=ALU.bypass, op1=ALU.mult,
                            accum_out=dotp[:, k, blk:blk + 1])
                        nc.scalar.activation(
                            out=st["trash"][:, blk, :], in_=p_ps[:, blk, :],
                            func=AF.Square,
                            accum_out=ssqp[:, k, blk:blk + 1])
                xp = sml.tile([128, NS, NBLK], F32, tag="xp")
                nc.vector.tensor_scalar(out=xp[:, 0:ns_], in0=ssqp[:, 0:ns_],
                                        scalar1=1.0 / D, scalar2=EPS_RMS,
                                        op0=ALU.mult, op1=ALU.add)
                rmsp = _newton_rsqrt(nc, nwp, xp, (NS, NBLK), iters=1)
                lp = sml.tile([128, NS, NBLK], F32, tag="lp")
                nc.vector.tensor_mul(lp[:, 0:ns_], dotp[:, 0:ns_],
                                     rmsp[:, 0:ns_])
                nc.scalar.activation(out=E_T[:, 0:ns_, :, nsrc],
                                     in_=lp[:, 0:ns_],
                                     func=AF.Tanh, scale=0.5)

            Ev = E_T[:, 0:ns_, :, 0:n]
            A = sml.tile([128, NS, NBLK, 5], F32, tag="A")
            nc.vector.tensor_scalar_add(A[:, 0:ns_, :, 0:n], Ev, 1.0)
            Bt = sml.tile([128, NS, NBLK, 5], F32, tag="B")
            nc.vector.tensor_scalar(out=Bt[:, 0:ns_, :, 0:n], in0=Ev,
                                    scalar1=-1.0, scalar2=-1.0,
                                    op0=ALU.mult, op1=ALU.subtract)
            R = sml.tile([128, NS, NBLK, 5], F32, tag="R")
            nc.vector.reciprocal(R[:, 0:ns_, :, 0:n], Bt[:, 0:ns_, :, 0:n])
            E = sml.tile([128, NS, NBLK, 5], F32, tag="E")
            nc.vector.tensor_mul(E[:, 0:ns_, :, 0:n], A[:, 0:ns_, :, 0:n],
                                 R[:, 0:ns_, :, 0:n])
            den = sml.tile([128, NS, NBLK], F32, tag="den")
            nc.vector.tensor_reduce(den[:, 0:ns_], E[:, 0:ns_, :, 0:n],
                                    axis=AX.X, op=ALU.add)
            rd = sml.tile([128, NS, NBLK], F32, tag="rd")
            nc.vector.reciprocal(rd[:, 0:ns_], den[:, 0:ns_])
            wts = sml.tile([128, NS, NBLK, 5], F32, tag="wts")
            rd_ap = rd[:, 0:ns_]
            nc.vector.scalar_tensor_tensor(
                out=wts[:, 0:ns_, :, 0:n], in0=E[:, 0:ns_, :, 0:n], scalar=1.0,
                in1=_bcast(rd_ap, n), op0=ALU.bypass, op1=ALU.mult)

            last = l == L - 1
            hsum = sml.tile([128, NS, NBLK], F32, tag="hsum")
            hs = []
            for st in sts:
                k = st["k"]
                slots, partial_ps = st["slots"], st["partial_ps"]

                def wsrc(i, blk):
                    return slots[:, i, blk, :]

                h = big.tile([128, NBLK, D], F32 if last else F32R,
                             tag="h_f32" if last else "h", name="h")
                hs.append(h)
                for blk in range(NBLK):
                    if n == 1:
                        nc.vector.tensor_scalar(
                            out=h[:, blk, :], in0=wsrc(0, blk),
                            scalar1=wts[:, k, blk, 0:1], scalar2=0.0,
                            op0=ALU.mult, op1=ALU.add,
                            accum_out=hsum[:, k, blk:blk + 1])
                    else:
                        nc.vector.tensor_scalar(
                            out=h[:, blk, :], in0=wsrc(0, blk),
                            scalar1=wts[:, k, blk, 0:1], scalar2=None,
                            op0=ALU.mult)
                    for i in range(1, n):
                        nc.vector.scalar_tensor_tensor(
                            out=h[:, blk, :], in0=wsrc(i, blk),
                            scalar=wts[:, k, blk, i:i + 1],
                            in1=h[:, blk, :],
                            op0=ALU.mult, op1=ALU.add,
                            accum_out=(hsum[:, k, blk:blk + 1]
                                       if i == n - 1 else None))
                if dbg and k == 0:
                    nc.sync.dma_start(dbgW[l], wts[:, 0])
                    if not last:
                        nc.sync.dma_start(dbgH[l], h[:])
                if last:
                    nc.sync.dma_start(out_v[bass.ds(st["it"], 1)], h[:])
                    if dbg and k == 0:
                        nc.sync.dma_start(dbgS[:], st["sdots"])
            if last:
                return

            hssq = sml.tile([128, NS, NBLK], F32, tag="hssq")
            for st, h in zip(sts, hs):
                k = st["k"]
                for blk in range(NBLK):
                    nc.scalar.activation(
                        out=st["trash"][:, blk, :], in_=h[:, blk, :],
                        func=AF.Square,
                        accum_out=hssq[:, k, blk:blk + 1])
            m2 = sml.tile([128, NS, NBLK], F32, tag="m2")
            nc.vector.tensor_mul(m2[:, 0:ns_], hsum[:, 0:ns_], hsum[:, 0:ns_])
            t1 = sml.tile([128, NS, NBLK], F32, tag="t1")
            nc.vector.tensor_scalar(out=t1[:, 0:ns_], in0=hssq[:, 0:ns_],
                                    scalar1=1.0 / D, scalar2=EPS_LN,
                                    op0=ALU.mult, op1=ALU.add)
            xs2 = sml.tile([128, NS, NBLK], F32, tag="xs2")
            nc.vector.scalar_tensor_tensor(
                out=xs2[:, 0:ns_], in0=m2[:, 0:ns_], scalar=-1.0 / (D * D),
                in1=t1[:, 0:ns_], op0=ALU.mult, op1=ALU.add)
            s_ln = _newton_rsqrt(nc, nwp, xs2, (NS, NBLK), iters=1)
            mu = sml.tile([128, NS, NBLK], F32, tag="mu")
            nc.vector.tensor_scalar_mul(mu[:, 0:ns_], hsum[:, 0:ns_], 1.0 / D)

            for st, h in zip(sts, hs):
                k = st["k"]
                slots, partial_ps = st["slots"], st["partial_ps"]
                xn = big.tile([128, NBLK, D], BF16, tag="xn", name="xn")
                for blk in range(NBLK):
                    nc.vector.tensor_scalar(
                        out=xn[:, blk, :], in0=h[:, blk, :],
                        scalar1=mu[:, k, blk:blk + 1],
                        scalar2=s_ln[:, k, blk:blk + 1],
                        op0=ALU.subtract, op1=ALU.mult)
                xnT_ps = pp_xnT.tile([128, F], BF16, tag="xnT_ps", name="xnT_ps")
                for blk in range(NBLK):
                    nc.tensor.matmul(xnT_ps[:, blk * 128:(blk + 1) * 128],
                                     xn[:, blk, :], ident16[:],
                                     is_transpose=True, start=True, stop=True,
                                     skip_group_check=True)
                xnT = big.tile([128, F], BF16, tag="xnT", name="xnT")
                nc.vector.tensor_copy(xnT[:], xnT_ps[:])
                G = []
                for half in range(2):
                    h1 = pp_big.tile([128, F], F32, tag="big_ps", name="h1")
                    nc.tensor.matmul(h1[:], w1p_sb[:, l, half, :], xnT[:],
                                     start=True, stop=True,
                                     skip_group_check=True)
                    gh = big.tile([128, F], BF16, tag=f"g{half}", name="gh")
                    nc.scalar.activation(
                        gh[:], h1[:], AF.Gelu,
                        bias=b1p_sb[:, 2 * l + half:2 * l + half + 1])
                    G.append(gh)
                vT_ps = pp_big.tile([128, F], F32, tag="big_ps", name="vT_ps")
                for kh in range(2):
                    nc.tensor.matmul(vT_ps[:], w2p_sb[:, l, kh, :], G[kh][:],
                                     start=(kh == 0), stop=(kh == 1),
                                     skip_group_check=True)
                vtT = big.tile([128, F], F32, tag="vtT", name="vtT")
                nc.vector.tensor_copy(vtT[:], vT_ps[:])
                for blk in range(NBLK):
                    nc.tensor.matmul(partial_ps[:, blk, :],
                                     vtT[:, blk * 128:(blk + 1) * 128],
                                     identf[:], is_transpose=True,
                                     start=(j == 0 and blk == 0),
                                     stop=(j == GROUP - 1 or l == L - 2),
                                     skip_group_check=True)
                nc.vector.tensor_copy(slots[:, g + 1, :, :], partial_ps[:])
                if j == GROUP - 1:
                    creation_pre(st, slots[:, g + 1, :, :], g + 1, sh)
            if j == GROUP - 1:
                creation_finish(sts, g + 1, sh)

        NS = 4 if tiles_per_core % 4 == 0 else (
            2 if tiles_per_core % 2 == 0 else 1)
        spd = es.enter_context(tc.tile_pool(name="spd", bufs=1))
        with tc.For_i(0, tiles_per_core // NS, 1,
              hint_engines=(mybir.EngineType.DVE,
                            mybir.EngineType.Activation,
                            mybir.EngineType.PE)) as it0:
            sh = {}
            sh["sdots_all"] = spd.tile([128, NS, 5, NBLK, L], F32,
                                       tag="sdots_all", name="sdots_all")
            sh["stats_sb_all"] = spd.tile([128, NS, NBLK, 33], F32,
                                          tag="stats_sb_all", name="stats_all")
            sts = [tile_start(it0 * NS + k, k, sh) for k in range(NS)]
            for st in sts:
                creation_pre(st, st["slots"][:, 0, :, :], 0, sh)
            creation_finish(sts, 0, sh)
            for l in range(L):
                emit_layer_quad(sts, l, sh)

    nc.finalize()
    return nc


def _prep_consts(w, ln_g, ln_b, W1, b1, W2):
    bf = ml_dtypes.bfloat16
    W1p = ln_g[:, :, None] * W1                                   # diag(g) @ W1
    b1p = b1 + np.einsum("ld,ldm->lm", ln_b, W1)                  # b1 + ln_b @ W1
    w1p = np.ascontiguousarray(W1p.transpose(1, 0, 2)).reshape(D, L * 2 * 128)
    b1p_sb = b1p.reshape(L, 2, 128).transpose(2, 0, 1).reshape(128, 2 * L)
    w2p = W2.reshape(L, 2, 128, D).transpose(2, 0, 1, 3)
    w2p = np.ascontiguousarray(w2p).reshape(128, L * 2 * D)
    return {
        "w_t": np.ascontiguousarray(w.T).astype(np.float32),
        "w_f": np.ascontiguousarray(w).astype(np.float32),
        "w1p": w1p.astype(bf),
        "b1p": np.ascontiguousarray(b1p_sb).astype(np.float32),
        "w2p": w2p.astype(bf),
    }


def kernel(embedding, w, ln_g, ln_b, W1, b1, W2, b2, _tiles=16, _trace=False, _dbg=False):
    if _trace:
        _install_ntff_hook()
    B, T, Dd = embedding.shape
    assert Dd == D
    n_tok = _tiles * F

    key = ("k", _tiles, bool(_dbg))
    if key not in _CACHE:
        _CACHE[key] = build(_tiles, dbg=_dbg)
    nc = _CACHE[key]

    assert np.all(np.asarray(b2) == 0.0), "nonzero b2 unsupported"
    consts = _prep_consts(np.asarray(w, np.float32),
                          np.asarray(ln_g, np.float32),
                          np.asarray(ln_b, np.float32),
                          np.asarray(W1, np.float32),
                          np.asarray(b1, np.float32),
                          np.asarray(W2, np.float32))
    emb_full = np.asarray(embedding, np.float32).reshape(B * T, D)

    per_core = B * T // N_CORES
    in_maps = []
    for c in range(N_CORES):
        shard = emb_full[c * per_core:(c + 1) * per_core][:n_tok]
        in_maps.append({"emb": np.ascontiguousarray(shard), **consts})

    res = run_bass_kernel_spmd(nc, in_maps, core_ids=list(range(N_CORES)),
                               trace=_trace)
    outs = [res.results[c]["out"] for c in range(N_CORES)]
    if _dbg:
        kernel.dbg = {k: res.results[0][k] for k in ("dbgH", "dbgW", "dbgS")}
    full = np.stack(outs).reshape(N_CORES, n_tok, D)
    kernel.last_exec_ns = getattr(res, "exec_time_ns", None)
    kernel.last_mean_ns = getattr(res, "mean_exec_time_ns", None)
    if n_tok == per_core:
        return full.reshape(B, T, D)
    return full  # debug partial run
